# revision 5
# baseline (speedup 1.0000x reference)
"""Sparse-attention Bass kernel for 8 TRN2 NeuronCores.

Sharding: query-row parallel. Core c owns query rows [c*512, (c+1)*512) of
BOTH batch elements. The [n, n] mask is row-sharded, K/V are computed
redundantly per core from the full batch.

Elementwise strategy (the baseline was exp/mask-bound on ACT+DVE):
  * Q weights are pre-scaled on host by A = 2^7*log2(e)*scale, so PSUM scores
    arrive as s' with exp(s*scale) = 2^(s'/128).
  * Route R2 (DVE): one fused scalar_tensor_tensor per tile computes
    (s' + B) * mask -> int16 (Schraudolph bitcast: int16 pattern read as bf16
    is the masked exp). One DVE op replaces ACT-exp + DVE-mask.
  * Route R3 (PE+ACT): a diagonal matmul injects +BIG*mask into the score
    PSUM (eye*BIG as fp8e5 lhsT, mask fp8e5 rhs), then one ACT exp with
    bias=-BIG*ln2/128 finishes the tile (exact fp32 cancellation for kept
    elements, exp(-177) == 0 for masked ones). No DVE op at all.
  Tiles are routed R3 on h2==1 (where no deferred-QKV PE work lives),
  R2 on h2==0, balancing ACT vs DVE vs PE.
  The Schraudolph constant C is centered (-6.25) so the trick's mean
  inflation (2.82%) cancels against the true-exp route in the softmax.

QKV/proj PSUM evacuation copies run on ACT (scalar.copy) to keep DVE free.
"""

import numpy as np
from contextlib import ExitStack

import concourse.bass as bass
import concourse.tile as tile
from concourse import bacc, mybir
from concourse.bass_utils import run_bass_kernel_spmd

BF16 = mybir.dt.bfloat16
F32 = mybir.dt.float32
I16 = mybir.dt.int16
FP8 = mybir.dt.float8e5
NPBF16 = mybir.dt.np(BF16)
NPFP8 = mybir.dt.np(FP8)

B, N, DIM, H, D = 2, 4096, 512, 16, 32
NCORES = 8
NQ = N // NCORES            # query rows per core per batch elem (512)
G = 4                       # head groups (4 heads each)
HG = H // G                 # heads per group (4)
JB = N // 128               # key blocks (32)
SCALE = float(D) ** -0.5

# Schraudolph / exp constants
A_PRE = 128.0 * np.log2(np.e) * SCALE          # folded into Q weights on host
C_CENTER = -6.25                                # centers trick vs true exp
B_CONST = float(127.0 * 128.0 + C_CENTER)       # STT additive constant
SC = float(np.float32(np.log(2.0) / 128.0))     # ACT exp scale (undoes A_PRE)
BIG = 32768.0                                   # mask injection magnitude
BIAS_NEG = float(-(np.float32(SC) * np.float32(BIG)))  # exact fp32 cancel
R3_JB = 28                                      # R3 tiles per (g, h2==1)

_CACHE = {}


def build_nc():
    nc = bacc.Bacc("TRN2", target_bir_lowering=False, debug=False)

    batT = nc.declare_dram_parameter("batt", [B, DIM, N], BF16, isOutput=False)
    qrT = nc.declare_dram_parameter("qrt", [B, DIM, NQ], BF16, isOutput=False)
    wqkv = nc.declare_dram_parameter("wqkv", [DIM, 3 * DIM], BF16, isOutput=False)
    wproj = nc.declare_dram_parameter("wproj", [DIM, DIM], BF16, isOutput=False)
    maskT = nc.declare_dram_parameter("maskt", [N, NQ], FP8, isOutput=False)
    eyeb = nc.declare_dram_parameter("eyeb", [128, 128], FP8, isOutput=False)
    out = nc.declare_dram_parameter("out", [B, NQ, DIM], F32, isOutput=True)

    Exp = mybir.ActivationFunctionType.Exp
    ADD = mybir.AluOpType.add
    MULT = mybir.AluOpType.mult

    with tile.TileContext(nc) as tc, ExitStack() as ctx:
        persist = ctx.enter_context(tc.tile_pool(name="persist", bufs=1))
        bpool = ctx.enter_context(tc.tile_pool(name="bpool", bufs=1))
        esbp = ctx.enter_context(tc.tile_pool(name="esbp", bufs=6))
        small = ctx.enter_context(tc.tile_pool(name="small", bufs=4))
        outp = ctx.enter_context(tc.tile_pool(name="outp", bufs=2))

        # ---- persistent loads -------------------------------------------
        wq_sb = []
        for k in range(4):
            t = persist.tile([128, 3 * DIM], BF16, tag=f"wqkv{k}")
            nc.sync.dma_start(out=t, in_=wqkv[k * 128:(k + 1) * 128, :])
            wq_sb.append(t)
        wp_sb = []
        for k in range(4):
            t = persist.tile([128, DIM], BF16, tag=f"wproj{k}")
            nc.sync.dma_start(out=t, in_=wproj[k * 128:(k + 1) * 128, :])
            wp_sb.append(t)
        mask_sb = []
        for jb in range(JB):
            t = persist.tile([128, NQ], FP8, tag=f"mask{jb}")
            nc.sync.dma_start(out=t, in_=maskT[jb * 128:(jb + 1) * 128, :])
            mask_sb.append(t)
        eye_sb = persist.tile([128, 128], FP8, tag="eyeb")
        nc.sync.dma_start(out=eye_sb, in_=eyeb[:, :])
        bias_sb = persist.tile([128, 1], F32, tag="biasneg")
        nc.vector.memset(bias_sb, BIAS_NEG)

        for b in range(B):
            # ---- QKV phase ----------------------------------------------
            batT_sb = []
            for k in range(4):
                t = bpool.tile([128, N], BF16, tag=f"batT{k}")
                nc.sync.dma_start(out=t, in_=batT[b, k * 128:(k + 1) * 128, :])
                batT_sb.append(t)
            qrT_sb = []
            for k in range(4):
                t = bpool.tile([128, NQ], BF16, tag=f"qrT{k}")
                nc.sync.dma_start(out=t, in_=qrT[b, k * 128:(k + 1) * 128, :])
                qrT_sb.append(t)

            def make_kt_chunk(pool, t, g, jc):
                ps = pool.tile([128, 512], F32, tag="xps")
                for k in range(4):
                    nc.tensor.matmul(
                        ps,
                        wq_sb[k][:, DIM + 128 * g: DIM + 128 * g + 128],
                        batT_sb[k][:, jc * 512:(jc + 1) * 512],
                        start=(k == 0), stop=(k == 3),
                    )
                nc.scalar.copy(t[:, jc * 512:(jc + 1) * 512], ps)

            def make_qt(pool, t, g):
                ps = pool.tile([128, 512], F32, tag="xps")
                for k in range(4):
                    nc.tensor.matmul(
                        ps,
                        wq_sb[k][:, 128 * g: 128 * g + 128],
                        qrT_sb[k],
                        start=(k == 0), stop=(k == 3),
                    )
                nc.scalar.copy(t, ps)

            def make_v(pool, t, nb):
                ps = pool.tile([128, 512], F32, tag="xps")
                for k in range(4):
                    nc.tensor.matmul(
                        ps,
                        batT_sb[k][:, nb * 128:(nb + 1) * 128],
                        wq_sb[k][:, 2 * DIM: 3 * DIM],
                        start=(k == 0), stop=(k == 3),
                    )
                dst = bass.AP(
                    tensor=t.tensor, offset=t.offset,
                    ap=[t.ap[0], [33, H], [1, D]],
                )
                nc.scalar.copy(dst, ps)
                ones = bass.AP(
                    tensor=t.tensor, offset=t.offset + D,
                    ap=[t.ap[0], [33, H]],
                )
                nc.vector.memset(ones, 1.0)

            kt_sb = [bpool.tile([128, N], BF16, tag=f"kt{g}", name=f"kt{g}")
                     for g in range(G)]
            qt_sb = [bpool.tile([128, NQ], BF16, tag=f"qt{g}", name=f"qt{g}")
                     for g in range(G)]
            v_sb = [bpool.tile([128, H * (D + 1)], BF16, tag=f"v{nb}", name=f"v{nb}")
                    for nb in range(JB)]
            with tc.tile_pool(name=f"mm{b}", bufs=2, space="PSUM") as mm_ps:
                for jc in range(N // 512):
                    make_kt_chunk(mm_ps, kt_sb[0], 0, jc)
                make_qt(mm_ps, qt_sb[0], 0)
                make_v(mm_ps, v_sb[0], 0)

            # ---- attention ----------------------------------------------
            with (tc.tile_pool(name=f"st{b}", bufs=2, space="PSUM") as st_ps,
                  tc.tile_pool(name=f"avp{b}", bufs=1, space="PSUM") as av_ps,
                  tc.tile_pool(name=f"xtr{b}", bufs=2, space="PSUM") as xtr_ps):
                pre_sb = []
                for g in range(G):
                    avh = [av_ps.tile([128, 512], F32, tag="av", name=f"av{h2}")
                           for h2 in range(2)]
                    for h2 in range(2):
                        av = avh[h2]
                        for jb in range(JB):
                            # R3 (PE-inject + ACT exp) on "light" slots only
                            # (no deferred-QKV PE work), alternating with R2
                            # (DVE fused STT) so ACT and DVE pipeline.
                            heavy = (h2 == 0 and (g == 0 or (g == 1 and jb < 10)))
                            r3 = (not heavy) and (jb % 2 == (g + h2) % 2)
                            st = st_ps.tile([128, 1024], F32, tag="st")
                            for rr in range(2):
                                r = 2 * h2 + rr
                                nc.tensor.matmul(
                                    st[:, rr * 512:rr * 512 + 512],
                                    kt_sb[g][32 * r:32 * r + 32, jb * 128:(jb + 1) * 128],
                                    qt_sb[g][32 * r:32 * r + 32, :],
                                    start=True, stop=not r3,
                                    tile_position=(32 * r, 0),
                                )
                            if h2 == 0:
                                if g == 0:
                                    # deferred QKV work, hidden under attention
                                    if jb < 16:
                                        make_kt_chunk(xtr_ps, kt_sb[1 + jb // 8], 1 + jb // 8, jb % 8)
                                    elif jb == 16:
                                        make_qt(xtr_ps, qt_sb[1], 1)
                                    if jb < JB - 1:
                                        make_v(xtr_ps, v_sb[jb + 1], jb + 1)
                                elif g == 1:
                                    if jb < 8:
                                        make_kt_chunk(xtr_ps, kt_sb[3], 3, jb)
                                    elif jb == 8:
                                        make_qt(xtr_ps, qt_sb[2], 2)
                                    elif jb == 9:
                                        make_qt(xtr_ps, qt_sb[3], 3)
                            e = esbp.tile([128, 1024], BF16, tag="e")
                            if r3:
                                # inject +BIG*mask into both banks (col-band
                                # packed: 2 concurrent M=64 matmuls per bank)
                                for rr in range(2):
                                    for cb in range(2):
                                        nc.tensor.matmul(
                                            st[cb * 64:(cb + 1) * 64, rr * 512:rr * 512 + 512],
                                            eye_sb[:, cb * 64:(cb + 1) * 64],
                                            mask_sb[jb],
                                            start=False, stop=(cb == 1),
                                            tile_position=(0, cb * 64),
                                            skip_group_check=True,
                                        )
                                nc.scalar.activation(e, st, Exp, scale=SC, bias=bias_sb)
                            else:
                                mrep = bass.AP(
                                    tensor=mask_sb[jb].tensor, offset=mask_sb[jb].offset,
                                    ap=[mask_sb[jb].ap[0], [0, 2], [1, 512]],
                                )
                                nc.vector.scalar_tensor_tensor(
                                    out=e.bitcast(I16), in0=st, scalar=B_CONST,
                                    in1=mrep, op0=ADD, op1=MULT,
                                )
                            for rr in range(2):
                                r = 2 * h2 + rr
                                h = g * HG + r
                                nc.tensor.matmul(
                                    av[64 * rr:64 * rr + 33, 0:512],
                                    v_sb[jb][:, 33 * h: 33 * h + 33],
                                    e[:, rr * 512:rr * 512 + 512],
                                    start=(jb == 0), stop=(jb == JB - 1),
                                    tile_position=(0, 64 * rr),
                                )
                    # normalize -> pre^T [128 (4h x 32d), NQ] bf16
                    pre = bpool.tile([128, NQ], BF16, tag=f"pre{g}")
                    for r in range(HG):
                        h2, rr = divmod(r, 2)
                        avx = avh[h2]
                        pb = 64 * rr
                        rsr = small.tile([1, NQ], F32, tag="rsr")
                        nc.scalar.copy(rsr, avx[pb + 32: pb + 33, 0:512])
                        rcp = small.tile([1, NQ], F32, tag="rcp")
                        nc.vector.reciprocal_approx_fast(rcp, rsr)
                        rcpb = small.tile([32, NQ], F32, tag="rcpb")
                        nc.gpsimd.partition_broadcast(rcpb, rcp[0:1, :], channels=32)
                        nc.vector.tensor_mul(
                            pre[32 * r: 32 * r + 32, :],
                            avx[pb: pb + 32, 0:512],
                            rcpb,
                        )
                    pre_sb.append(pre)

            # ---- output projection --------------------------------------
            with tc.tile_pool(name=f"pj{b}", bufs=2, space="PSUM") as mm_ps:
                for ib in range(NQ // 128):
                    ps = mm_ps.tile([128, DIM], F32)
                    for g in range(G):
                        nc.tensor.matmul(
                            ps,
                            pre_sb[g][:, ib * 128:(ib + 1) * 128],
                            wp_sb[g],
                            start=(g == 0), stop=(g == 3),
                        )
                    o = outp.tile([128, DIM], F32, tag="o")
                    nc.scalar.copy(o, ps)
                    nc.sync.dma_start(out=out[b, ib * 128:(ib + 1) * 128, :], in_=o)

    nc.compile()
    return nc


def _prep_inputs(batch, w_qkv, w_proj, custom_mask):
    batch = np.asarray(batch, np.float32)
    wqkv_f = np.asarray(w_qkv, np.float32).copy()
    wqkv_f[:, :DIM] *= np.float32(A_PRE)       # fold Schraudolph pre-scale into Q
    wqkv_bf = wqkv_f.astype(NPBF16)
    wproj_bf = np.asarray(w_proj, np.float32).astype(NPBF16)
    batT = np.ascontiguousarray(batch.transpose(0, 2, 1)).astype(NPBF16)
    m = np.asarray(custom_mask, np.float32)[0, 0]  # [N, N] 0/1
    eye = (np.eye(128, dtype=np.float32) * np.float32(BIG)).astype(NPFP8)
    in_maps = []
    for c in range(NCORES):
        rows = slice(c * NQ, (c + 1) * NQ)
        qrT = np.ascontiguousarray(batch[:, rows, :].transpose(0, 2, 1)).astype(NPBF16)
        mT = np.ascontiguousarray(m[rows, :].T).astype(NPFP8)
        in_maps.append({
            "batt": batT, "qrt": qrT, "wqkv": wqkv_bf,
            "wproj": wproj_bf, "maskt": mT, "eyeb": eye,
        })
    return in_maps


def _run(in_maps, trace=False, **kw):
    if "nc" not in _CACHE:
        _CACHE["nc"] = build_nc()
    return run_bass_kernel_spmd(
        _CACHE["nc"], in_maps, core_ids=list(range(NCORES)), trace=trace, **kw
    )


def kernel(batch, w_qkv, w_proj, custom_mask):
    in_maps = _prep_inputs(batch, w_qkv, w_proj, custom_mask)
    res = _run(in_maps)
    full = np.empty((B, N, DIM), np.float32)
    for c in range(NCORES):
        full[:, c * NQ:(c + 1) * NQ, :] = res.results[c]["out"]
    return full


# revision 8
# speedup vs baseline: 1.4348x; 1.4348x over previous
"""Sparse-attention Bass kernel for 8 TRN2 NeuronCores.

Sharding: query-row parallel. Core c owns query rows [c*512, (c+1)*512) of
BOTH batch elements. The [n, n] mask is row-sharded, K/V are computed
redundantly per core from the full batch.

Elementwise strategy (the baseline was exp/mask-bound on ACT+DVE):
  * Q weights are pre-scaled on host by A = 2^7*log2(e)*scale, so PSUM scores
    arrive as s' with exp(s*scale) = 2^(s'/128).
  * Route R2 (DVE): one fused scalar_tensor_tensor per tile computes
    (s' + B) * mask -> int16 (Schraudolph bitcast: int16 pattern read as bf16
    is the masked exp). One DVE op replaces ACT-exp + DVE-mask.
  * Route R3 (PE+ACT): a diagonal matmul injects +BIG*mask into the score
    PSUM (eye*BIG as fp8e5 lhsT, mask fp8e5 rhs), then one ACT exp with
    bias=-BIG*ln2/128 finishes the tile (exact fp32 cancellation for kept
    elements, exp(-177) == 0 for masked ones). No DVE op at all.
  Tiles are routed R3 on h2==1 (where no deferred-QKV PE work lives),
  R2 on h2==0, balancing ACT vs DVE vs PE.
  The Schraudolph constant C is centered (-6.25) so the trick's mean
  inflation (2.82%) cancels against the true-exp route in the softmax.

QKV/proj PSUM evacuation copies run on ACT (scalar.copy) to keep DVE free.
"""

import numpy as np
from contextlib import ExitStack

import concourse.bass as bass
import concourse.tile as tile
from concourse import bacc, mybir
from concourse.bass_utils import run_bass_kernel_spmd

BF16 = mybir.dt.bfloat16
F32 = mybir.dt.float32
I16 = mybir.dt.int16
FP8 = mybir.dt.float8e5
NPBF16 = mybir.dt.np(BF16)
NPFP8 = mybir.dt.np(FP8)

B, N, DIM, H, D = 2, 4096, 512, 16, 32
NCORES = 8
NQ = N // NCORES            # query rows per core per batch elem (512)
G = 4                       # head groups (4 heads each)
HG = H // G                 # heads per group (4)
JB = N // 128               # key blocks (32)
SCALE = float(D) ** -0.5

# Schraudolph / exp constants
A_PRE = 128.0 * np.log2(np.e) * SCALE          # folded into Q weights on host
C_CENTER = -6.25                                # centers trick vs true exp
B_CONST = float(127.0 * 128.0 + C_CENTER)       # STT additive constant
SC = float(np.float32(np.log(2.0) / 128.0))     # ACT exp scale (undoes A_PRE)
BIG = 32768.0                                   # mask injection magnitude
BIAS_NEG = float(-(np.float32(SC) * np.float32(BIG)))  # exact fp32 cancel
R3_JB = 28                                      # R3 tiles per (g, h2==1)

_CACHE = {}


def build_nc():
    nc = bacc.Bacc("TRN2", target_bir_lowering=False, debug=False)

    batT = nc.declare_dram_parameter("batt", [B, DIM, N], BF16, isOutput=False)
    qrT = nc.declare_dram_parameter("qrt", [B, DIM, NQ], BF16, isOutput=False)
    wqkv = nc.declare_dram_parameter("wqkv", [DIM, 3 * DIM], BF16, isOutput=False)
    wproj = nc.declare_dram_parameter("wproj", [DIM, DIM], BF16, isOutput=False)
    maskT = nc.declare_dram_parameter("maskt", [N, NQ], FP8, isOutput=False)
    eyeb = nc.declare_dram_parameter("eyeb", [128, 128], FP8, isOutput=False)
    out = nc.declare_dram_parameter("out", [B, NQ, DIM], F32, isOutput=True)

    Exp = mybir.ActivationFunctionType.Exp
    ADD = mybir.AluOpType.add
    MULT = mybir.AluOpType.mult

    with tile.TileContext(nc) as tc, ExitStack() as ctx:
        persist = ctx.enter_context(tc.tile_pool(name="persist", bufs=1))
        bpool = ctx.enter_context(tc.tile_pool(name="bpool", bufs=1))
        esbp = ctx.enter_context(tc.tile_pool(name="esbp", bufs=8))
        small = ctx.enter_context(tc.tile_pool(name="small", bufs=4))
        outp = ctx.enter_context(tc.tile_pool(name="outp", bufs=2))

        # ---- persistent loads -------------------------------------------
        wq_sb = []
        for k in range(4):
            t = persist.tile([128, 3 * DIM], BF16, tag=f"wqkv{k}")
            nc.sync.dma_start(out=t, in_=wqkv[k * 128:(k + 1) * 128, :])
            wq_sb.append(t)
        wp_sb = []
        for k in range(4):
            t = persist.tile([128, DIM], BF16, tag=f"wproj{k}")
            nc.sync.dma_start(out=t, in_=wproj[k * 128:(k + 1) * 128, :])
            wp_sb.append(t)
        mask_sb = []
        for jb in range(JB):
            t = persist.tile([128, NQ], FP8, tag=f"mask{jb}")
            nc.sync.dma_start(out=t, in_=maskT[jb * 128:(jb + 1) * 128, :])
            mask_sb.append(t)
        eye_sb = persist.tile([128, 128], FP8, tag="eyeb")
        nc.sync.dma_start(out=eye_sb, in_=eyeb[:, :])
        bias_sb = persist.tile([128, 1], F32, tag="biasneg")
        nc.vector.memset(bias_sb, BIAS_NEG)

        for b in range(B):
            # ---- QKV phase ----------------------------------------------
            batT_sb = []
            for k in range(4):
                t = bpool.tile([128, N], BF16, tag=f"batT{k}")
                nc.sync.dma_start(out=t, in_=batT[b, k * 128:(k + 1) * 128, :])
                batT_sb.append(t)
            qrT_sb = []
            for k in range(4):
                t = bpool.tile([128, NQ], BF16, tag=f"qrT{k}")
                nc.sync.dma_start(out=t, in_=qrT[b, k * 128:(k + 1) * 128, :])
                qrT_sb.append(t)

            def make_kt_chunk(pool, t, g, jc):
                ps = pool.tile([128, 512], F32, tag="xps")
                for k in range(4):
                    nc.tensor.matmul(
                        ps,
                        wq_sb[k][:, DIM + 128 * g: DIM + 128 * g + 128],
                        batT_sb[k][:, jc * 512:(jc + 1) * 512],
                        start=(k == 0), stop=(k == 3),
                    )
                nc.scalar.copy(t[:, jc * 512:(jc + 1) * 512], ps)

            def make_qt(pool, t, g):
                ps = pool.tile([128, 512], F32, tag="xps")
                for k in range(4):
                    nc.tensor.matmul(
                        ps,
                        wq_sb[k][:, 128 * g: 128 * g + 128],
                        qrT_sb[k],
                        start=(k == 0), stop=(k == 3),
                    )
                nc.scalar.copy(t, ps)

            def make_v(pool, t, nb):
                ps = pool.tile([128, 512], F32, tag="xps")
                for k in range(4):
                    nc.tensor.matmul(
                        ps,
                        batT_sb[k][:, nb * 128:(nb + 1) * 128],
                        wq_sb[k][:, 2 * DIM: 3 * DIM],
                        start=(k == 0), stop=(k == 3),
                    )
                dst = bass.AP(
                    tensor=t.tensor, offset=t.offset,
                    ap=[t.ap[0], [33, H], [1, D]],
                )
                nc.scalar.copy(dst, ps)
                ones = bass.AP(
                    tensor=t.tensor, offset=t.offset + D,
                    ap=[t.ap[0], [33, H]],
                )
                nc.vector.memset(ones, 1.0)

            kt_sb = [bpool.tile([128, N], BF16, tag=f"kt{g}", name=f"kt{g}")
                     for g in range(G)]
            qt_sb = [bpool.tile([128, NQ], BF16, tag=f"qt{g}", name=f"qt{g}")
                     for g in range(G)]
            v_sb = [bpool.tile([128, H * (D + 1)], BF16, tag=f"v{nb}", name=f"v{nb}")
                    for nb in range(JB)]
            with tc.tile_pool(name=f"mm{b}", bufs=2, space="PSUM") as mm_ps:
                for jc in range(N // 512):
                    make_kt_chunk(mm_ps, kt_sb[0], 0, jc)
                make_qt(mm_ps, qt_sb[0], 0)
                make_v(mm_ps, v_sb[0], 0)

            # ---- attention ----------------------------------------------
            with (tc.tile_pool(name=f"st{b}", bufs=3, space="PSUM") as st_ps,
                  tc.tile_pool(name=f"avp{b}", bufs=1, space="PSUM") as av_ps,
                  tc.tile_pool(name=f"xtr{b}", bufs=1, space="PSUM") as xtr_ps):
                pre_sb = []
                for g in range(G):
                    pre = bpool.tile([128, NQ], BF16, tag=f"pre{g}")
                    for h2 in range(2):
                        av = av_ps.tile([128, 512], F32, tag="av", name=f"av{h2}")
                        for jb in range(JB):
                            # R3 (PE-inject + ACT exp) on "light" slots only
                            # (no deferred-QKV PE work), alternating with R2
                            # (DVE fused STT) so ACT and DVE pipeline.
                            heavy = (h2 == 0 and (g == 0 or (g == 1 and jb < 10)))
                            r3 = (not heavy) and (jb % 2 == (g + h2) % 2)
                            st = st_ps.tile([128, 1024], F32, tag="st")
                            for rr in range(2):
                                r = 2 * h2 + rr
                                nc.tensor.matmul(
                                    st[:, rr * 512:rr * 512 + 512],
                                    kt_sb[g][32 * r:32 * r + 32, jb * 128:(jb + 1) * 128],
                                    qt_sb[g][32 * r:32 * r + 32, :],
                                    start=True, stop=not r3,
                                    tile_position=(32 * r, 0),
                                )
                            if h2 == 0:
                                if g == 0:
                                    # deferred QKV work, hidden under attention
                                    if jb < 16:
                                        make_kt_chunk(xtr_ps, kt_sb[1 + jb // 8], 1 + jb // 8, jb % 8)
                                    elif jb == 16:
                                        make_qt(xtr_ps, qt_sb[1], 1)
                                    if jb < JB - 1:
                                        make_v(xtr_ps, v_sb[jb + 1], jb + 1)
                                elif g == 1:
                                    if jb < 8:
                                        make_kt_chunk(xtr_ps, kt_sb[3], 3, jb)
                                    elif jb == 8:
                                        make_qt(xtr_ps, qt_sb[2], 2)
                                    elif jb == 9:
                                        make_qt(xtr_ps, qt_sb[3], 3)
                            e = esbp.tile([128, 1024], BF16, tag="e")
                            if r3:
                                # inject +BIG*mask into both banks (col-band
                                # packed: 2 concurrent M=64 matmuls per bank)
                                for rr in range(2):
                                    for cb in range(2):
                                        nc.tensor.matmul(
                                            st[cb * 64:(cb + 1) * 64, rr * 512:rr * 512 + 512],
                                            eye_sb[:, cb * 64:(cb + 1) * 64],
                                            mask_sb[jb],
                                            start=False, stop=(cb == 1),
                                            tile_position=(0, cb * 64),
                                            skip_group_check=True,
                                        )
                                nc.scalar.activation(e, st, Exp, scale=SC, bias=bias_sb)
                            else:
                                mrep = bass.AP(
                                    tensor=mask_sb[jb].tensor, offset=mask_sb[jb].offset,
                                    ap=[mask_sb[jb].ap[0], [0, 2], [1, 512]],
                                )
                                nc.vector.scalar_tensor_tensor(
                                    out=e.bitcast(I16), in0=st, scalar=B_CONST,
                                    in1=mrep, op0=ADD, op1=MULT,
                                )
                            for rr in range(2):
                                r = 2 * h2 + rr
                                h = g * HG + r
                                nc.tensor.matmul(
                                    av[64 * rr:64 * rr + 33, 0:512],
                                    v_sb[jb][:, 33 * h: 33 * h + 33],
                                    e[:, rr * 512:rr * 512 + 512],
                                    start=(jb == 0), stop=(jb == JB - 1),
                                    tile_position=(0, 64 * rr),
                                )
                        # normalize this h2's two heads -> pre^T rows; frees
                        # the av bank so the next h2 can rotate into it
                        for rr in range(2):
                            r = 2 * h2 + rr
                            pb = 64 * rr
                            rsr = small.tile([1, NQ], F32, tag="rsr")
                            nc.scalar.copy(rsr, av[pb + 32: pb + 33, 0:512])
                            rcp = small.tile([1, NQ], F32, tag="rcp")
                            nc.vector.reciprocal_approx_fast(rcp, rsr)
                            rcpb = small.tile([32, NQ], F32, tag="rcpb")
                            nc.gpsimd.partition_broadcast(rcpb, rcp[0:1, :], channels=32)
                            nc.vector.tensor_mul(
                                pre[32 * r: 32 * r + 32, :],
                                av[pb: pb + 32, 0:512],
                                rcpb,
                            )
                    pre_sb.append(pre)

            # ---- output projection --------------------------------------
            with tc.tile_pool(name=f"pj{b}", bufs=2, space="PSUM") as mm_ps:
                for ib in range(NQ // 128):
                    ps = mm_ps.tile([128, DIM], F32)
                    for g in range(G):
                        nc.tensor.matmul(
                            ps,
                            pre_sb[g][:, ib * 128:(ib + 1) * 128],
                            wp_sb[g],
                            start=(g == 0), stop=(g == 3),
                        )
                    o = outp.tile([128, DIM], F32, tag="o")
                    nc.scalar.copy(o, ps)
                    nc.sync.dma_start(out=out[b, ib * 128:(ib + 1) * 128, :], in_=o)

    nc.compile()
    return nc


def _prep_inputs(batch, w_qkv, w_proj, custom_mask):
    batch = np.asarray(batch, np.float32)
    wqkv_f = np.asarray(w_qkv, np.float32).copy()
    wqkv_f[:, :DIM] *= np.float32(A_PRE)       # fold Schraudolph pre-scale into Q
    wqkv_bf = wqkv_f.astype(NPBF16)
    wproj_bf = np.asarray(w_proj, np.float32).astype(NPBF16)
    batT = np.ascontiguousarray(batch.transpose(0, 2, 1)).astype(NPBF16)
    m = np.asarray(custom_mask, np.float32)[0, 0]  # [N, N] 0/1
    eye = (np.eye(128, dtype=np.float32) * np.float32(BIG)).astype(NPFP8)
    in_maps = []
    for c in range(NCORES):
        rows = slice(c * NQ, (c + 1) * NQ)
        qrT = np.ascontiguousarray(batch[:, rows, :].transpose(0, 2, 1)).astype(NPBF16)
        mT = np.ascontiguousarray(m[rows, :].T).astype(NPFP8)
        in_maps.append({
            "batt": batT, "qrt": qrT, "wqkv": wqkv_bf,
            "wproj": wproj_bf, "maskt": mT, "eyeb": eye,
        })
    return in_maps


def _run(in_maps, trace=False, **kw):
    if "nc" not in _CACHE:
        _CACHE["nc"] = build_nc()
    return run_bass_kernel_spmd(
        _CACHE["nc"], in_maps, core_ids=list(range(NCORES)), trace=trace, **kw
    )


def kernel(batch, w_qkv, w_proj, custom_mask):
    in_maps = _prep_inputs(batch, w_qkv, w_proj, custom_mask)
    res = _run(in_maps)
    full = np.empty((B, N, DIM), np.float32)
    for c in range(NCORES):
        full[:, c * NQ:(c + 1) * NQ, :] = res.results[c]["out"]
    return full
